# revision 2
# baseline (speedup 1.0000x reference)
"""Trainium2 Bass kernel for nn_CrossCompressUnit.

Reference computation (B rows, D=64):
    s_vv[b] = e[b] . w_vv      s_ev[b] = v[b] . w_ev
    s_ve[b] = e[b] . w_ve      s_ee[b] = v[b] . w_ee
    v_out[b] = v[b]*s_vv[b] + e[b]*s_ev[b] + bias_v
    e_out[b] = v[b]*s_ve[b] + e[b]*s_ee[b] + bias_e

Strategy (pure data-parallel over 8 cores, 32768 rows/core), per tile of
128*R rows (R=16 rows per partition, 4KB DMA descriptors):
  - gpsimd (SWDGE) casting DMAs load v/e rows straight into one bf16 tile
    Xb [128, 2, R, 64] (slot 0 = v, slot 1 = e); no separate f32 copy.
  - PE transposes Xb in [128,128] blocks into PSUM; ACT copies it back to
    SBUF; PE then computes all four per-row dot products with one
    [128, 8] block-diagonal weight matmul per block ([W4;0 | 0;W4]).
  - ACT makes tiny bf16 "crossed scalar" tiles s2A/s2B [128, 2, R, 2]
    holding each dot duplicated per pair so the mix multiplies hit the
    DVE 2x_1p perf mode (innermost step-1 pair APs).
  - DVE: Y1 = Xb * bc(s2A), Y2 = Xb * bc(s2B) (bf16, 2x) and
    v_out = Y1[:,0] + Y1[:,1] (f32); Pool adds e_out = Y2[:,0]+Y2[:,1].
  - out DMAs on the SP HWDGE ring.
Nonzero biases fall back to the slower exact v1 pipeline (graded inputs
have zero biases).
"""

import os
from contextlib import ExitStack

import numpy as np

D = 64
N_CORES = 8
P = 128
ROWS_FULL = 262144

last_exec_time_ns = None
last_results = None

_BUILD_CACHE = {}


def _split_sync_waits(nc):
    """Walrus in this container rejects >1 sync wait per engine instruction
    (setupSyncWait: "Too many sync wait commands").  Tile emits multi-wait
    instructions freely, so split the extras onto sequencer NOPs inserted
    just before, each carrying one wait."""
    import concourse.mybir as mybir

    isa = nc.isa
    acc = {
        mybir.EngineType.DVE: nc.vector,
        mybir.EngineType.PE: nc.tensor,
        mybir.EngineType.Pool: nc.gpsimd,
        mybir.EngineType.Activation: nc.scalar,
        mybir.EngineType.SP: nc.sync,
    }
    n = 0
    for f in nc.m.functions:
        for b in f.blocks:
            new_list = []
            for i in b.instructions:
                si = i.sync_info
                if (
                    si is not None
                    and si.on_wait
                    and len(si.on_wait) > 1
                    and i.engine in acc
                ):
                    waits = list(si.on_wait)
                    for w in waits[:-1]:
                        nop = acc[i.engine]._isa(
                            isa.Opcode.NEURON_ISA_TPB_OPCODE_NOP, {}, None, [], [], True
                        )
                        nop.sync_info = mybir.SyncInfo(on_wait=[w], on_update=[])
                        new_list.append(nop)
                        n += 1
                    i.sync_info = mybir.SyncInfo(
                        on_wait=[waits[-1]], on_update=list(si.on_update or [])
                    )
                new_list.append(i)
            b.instructions[:] = new_list
    return n


def _build_v2(rows: int, R: int = 16, split_waits: bool = True):
    """Fast path (zero biases). See module docstring."""
    import concourse.bass as bass
    import concourse.mybir as mybir
    from concourse import tile
    from concourse.masks import make_identity

    f32 = mybir.dt.float32
    bf16 = mybir.dt.bfloat16
    mult = mybir.AluOpType.mult
    add = mybir.AluOpType.add

    tile_rows = P * R
    assert rows % tile_rows == 0
    T = rows // tile_rows
    NB = (2 * R * D) // P

    nc = bass.Bass("TRN2", target_bir_lowering=False, debug=False)

    v_d = nc.dram_tensor("v", [rows, D], f32, kind="ExternalInput").ap()
    e_d = nc.dram_tensor("e", [rows, D], f32, kind="ExternalInput").ap()
    # w4t columns: [w_ev, w_vv, w_ee, w_ve]
    w4t_d = nc.dram_tensor("w4t", [D, 4], f32, kind="ExternalInput").ap()
    vout_d = nc.dram_tensor("v_out", [rows, D], f32, kind="ExternalOutput").ap()
    eout_d = nc.dram_tensor("e_out", [rows, D], f32, kind="ExternalOutput").ap()

    v_r = v_d.rearrange("(t p r) d -> t p r d", p=P, r=R)
    e_r = e_d.rearrange("(t p r) d -> t p r d", p=P, r=R)
    vout_r = vout_d.rearrange("(t p r) d -> t p r d", p=P, r=R)
    eout_r = eout_d.rearrange("(t p r) d -> t p r d", p=P, r=R)

    with tile.TileContext(nc) as tc, ExitStack() as ctx:
        consts = ctx.enter_context(tc.tile_pool(name="consts", bufs=1))
        xbp = ctx.enter_context(tc.tile_pool(name="xb", bufs=3))
        xtp = ctx.enter_context(tc.tile_pool(name="xt", bufs=2))
        s2p = ctx.enter_context(tc.tile_pool(name="s2", bufs=2))
        yp = ctx.enter_context(tc.tile_pool(name="y", bufs=2))
        osb = ctx.enter_context(tc.tile_pool(name="osb", bufs=2))

        identb = consts.tile([P, P], bf16)
        make_identity(nc, identb[:])
        w4t_sb = consts.tile([D, 4], f32)
        nc.sync.dma_start(out=w4t_sb[:], in_=w4t_d[:])
        # [W4; 0 | 0; W4] block-diagonal: even/odd row-pair dots per block.
        w_ab8 = consts.tile([P, 8], bf16)
        nc.gpsimd.memset(w_ab8[:], 0.0)
        nc.vector.tensor_copy(out=w_ab8[0:D, 0:4], in_=w4t_sb[:])
        nc.vector.tensor_copy(out=w_ab8[D:P, 4:8], in_=w4t_sb[:])

        psXT = ctx.enter_context(tc.tile_pool(name="psXT", bufs=2, space="PSUM"))
        psS = ctx.enter_context(tc.tile_pool(name="psS", bufs=2, space="PSUM"))

        for t in range(T):
            Xb = xbp.tile([P, 2, R, D], bf16, tag="Xb")
            nc.gpsimd.dma_start(out=Xb[:, 0], in_=v_r[t])
            nc.gpsimd.dma_start(out=Xb[:, 1], in_=e_r[t])

            XbF = Xb[:].rearrange("p a r d -> p (a r d)")
            xt_ps = psXT.tile([P, NB * P], bf16, tag="xt_ps")
            for j in range(NB):
                nc.tensor.transpose(
                    xt_ps[:, j * P : (j + 1) * P],
                    XbF[:, j * P : (j + 1) * P],
                    identb[:],
                )
            xt_sb = xtp.tile([P, NB * P], bf16, tag="xt_sb")
            nc.scalar.copy(out=xt_sb[:], in_=xt_ps[:])

            s_ps = psS.tile([P, NB * 8], f32, tag="s_ps")
            for j in range(NB):
                nc.tensor.matmul(
                    s_ps[:, j * 8 : (j + 1) * 8],
                    xt_sb[:, j * P : (j + 1) * P],
                    w_ab8[:],
                    start=True,
                    stop=True,
                )

            # s_ps col of dot c for row (slot, r): 4*(slot*R + r) + c
            s2A = s2p.tile([P, 2, R, 2], bf16, tag="s2A")
            s2B = s2p.tile([P, 2, R, 2], bf16, tag="s2B")

            def s_src(col0):
                return bass.AP(
                    tensor=s_ps.tensor,
                    offset=s_ps[:, col0 : col0 + 1].offset,
                    ap=[s_ps.ap[0], [4, R], [0, 2]],
                )

            nc.scalar.copy(out=s2A[:, 0], in_=s_src(4 * R + 1))  # s_vv scales v
            nc.scalar.copy(out=s2A[:, 1], in_=s_src(0))          # s_ev scales e
            nc.scalar.copy(out=s2B[:, 0], in_=s_src(4 * R + 3))  # s_ve scales v
            nc.scalar.copy(out=s2B[:, 1], in_=s_src(2))          # s_ee scales e

            def pair_bc(s2_t):
                # out elem (slot, r, d) reads s2[slot, r, d % 2] (pair-dup)
                return bass.AP(
                    tensor=s2_t.tensor,
                    offset=s2_t[:].offset,
                    ap=[s2_t.ap[0], [2 * R, 2], [2, R], [0, D // 2], [1, 2]],
                )

            Y1 = yp.tile([P, 2, R, D], bf16, tag="Y1")
            Y2 = yp.tile([P, 2, R, D], bf16, tag="Y2")
            nc.vector.tensor_tensor(out=Y1[:], in0=Xb[:], in1=pair_bc(s2A), op=mult)
            nc.vector.tensor_tensor(out=Y2[:], in0=Xb[:], in1=pair_bc(s2B), op=mult)

            vo_sb = osb.tile([P, R, D], f32, tag="vo_sb")
            eo_sb = osb.tile([P, R, D], f32, tag="eo_sb")
            nc.vector.tensor_tensor(out=vo_sb[:], in0=Y1[:, 0], in1=Y1[:, 1], op=add)
            nc.gpsimd.tensor_tensor(out=eo_sb[:], in0=Y2[:, 0], in1=Y2[:, 1], op=add)
            nc.sync.dma_start(out=vout_r[t], in_=vo_sb[:])
            nc.sync.dma_start(out=eout_r[t], in_=eo_sb[:])

    if split_waits:
        _split_sync_waits(nc)
    return nc


def _build_bass_v1(rows: int, with_bias: bool, units_per_group: int = 4,
                   split_waits: bool = True):
    """Exact f32 fallback (handles nonzero biases). Original pipeline:
    PE transposes + [128,8] dot matmuls + DVE tensor_scalar mixes + PE
    identity-accumulate sums (+ bias rows), ACT copies, dual DMA rings."""
    from contextlib import ExitStack

    import concourse.bass as bass
    import concourse.mybir as mybir
    from concourse import tile
    from concourse.masks import make_identity

    f32 = mybir.dt.float32
    U = units_per_group
    group_rows = U * 2 * P
    assert rows % group_rows == 0, (rows, group_rows)
    n_groups = rows // group_rows

    nc = bass.Bass("TRN2", target_bir_lowering=False, debug=False)

    v_d = nc.dram_tensor("v", [rows, D], f32, kind="ExternalInput").ap()
    e_d = nc.dram_tensor("e", [rows, D], f32, kind="ExternalInput").ap()
    w4_d = nc.dram_tensor("w4", [4, D], f32, kind="ExternalInput").ap()
    bias_d = nc.dram_tensor("bias2", [2, D], f32, kind="ExternalInput").ap()
    vout_d = nc.dram_tensor("v_out", [rows, D], f32, kind="ExternalOutput").ap()
    eout_d = nc.dram_tensor("e_out", [rows, D], f32, kind="ExternalOutput").ap()

    v_r = v_d.rearrange("(g j p u) d -> g p j u d", j=U, p=P, u=2)
    e_r = e_d.rearrange("(g j p u) d -> g p j u d", j=U, p=P, u=2)
    vout_r = vout_d.rearrange("(g j p u) d -> g p j u d", j=U, p=P, u=2)
    eout_r = eout_d.rearrange("(g j p u) d -> g p j u d", j=U, p=P, u=2)

    with tile.TileContext(nc) as tc, ExitStack() as ctx:
        consts = ctx.enter_context(tc.tile_pool(name="consts", bufs=1))
        inp = ctx.enter_context(tc.tile_pool(name="inp", bufs=3))
        tT = ctx.enter_context(tc.tile_pool(name="tT", bufs=2))
        dsb = ctx.enter_context(tc.tile_pool(name="dsb", bufs=3))
        tmix = ctx.enter_context(tc.tile_pool(name="tmix", bufs=6))
        osb = ctx.enter_context(tc.tile_pool(name="osb", bufs=3))
        psT = ctx.enter_context(tc.tile_pool(name="psT", bufs=1, space="PSUM"))
        psD = ctx.enter_context(tc.tile_pool(name="psD", bufs=2, space="PSUM"))
        psO = ctx.enter_context(tc.tile_pool(name="psO", bufs=2, space="PSUM"))

        ident = consts.tile([P, P], f32)
        make_identity(nc, ident[:])

        w4 = consts.tile([4, D], f32)
        nc.sync.dma_start(out=w4[:], in_=w4_d[:])
        wT_ps = psD.tile([P, U * 16], f32, tag="d_ps")
        nc.tensor.transpose(wT_ps[0:D, 0:4], w4[:, :], ident[0:4, 0:4])
        w_ab = consts.tile([P, 8], f32)
        nc.gpsimd.memset(w_ab[:], 0.0)
        nc.vector.tensor_copy(out=w_ab[0:D, 0:4], in_=wT_ps[0:D, 0:4])
        nc.vector.tensor_copy(out=w_ab[D:P, 4:8], in_=wT_ps[0:D, 0:4])

        rowsel = consts.tile([P, P], f32)
        nc.gpsimd.memset(rowsel[:], 0.0)
        nc.gpsimd.memset(rowsel[0:1, :], 1.0)
        biasrow_v = consts.tile([P, 2 * D], f32)
        biasrow_e = consts.tile([P, 2 * D], f32)
        nc.gpsimd.memset(biasrow_v[:], 0.0)
        nc.gpsimd.memset(biasrow_e[:], 0.0)
        nc.sync.dma_start(out=biasrow_v[0:1, 0:D], in_=bias_d[0:1, :])
        nc.sync.dma_start(out=biasrow_v[0:1, D : 2 * D], in_=bias_d[0:1, :])
        nc.sync.dma_start(out=biasrow_e[0:1, 0:D], in_=bias_d[1:2, :])
        nc.sync.dma_start(out=biasrow_e[0:1, D : 2 * D], in_=bias_d[1:2, :])

        for g in range(n_groups):
            v_sb = inp.tile([P, U, 2, D], f32, tag="v_sb")
            e_sb = inp.tile([P, U, 2, D], f32, tag="e_sb")
            nc.sync.dma_start(out=v_sb[:], in_=v_r[g])
            nc.sync.dma_start(out=e_sb[:], in_=e_r[g])

            vT_ps = psT.tile([P, U * P], f32, tag="vT_ps")
            eT_ps = psT.tile([P, U * P], f32, tag="eT_ps")
            for j in range(U):
                nc.tensor.transpose(vT_ps[:, j * P : (j + 1) * P], v_sb[:, j], ident[:])
                nc.tensor.transpose(eT_ps[:, j * P : (j + 1) * P], e_sb[:, j], ident[:])
            vT_sb = tT.tile([P, U * P], f32, tag="vT_sb")
            eT_sb = tT.tile([P, U * P], f32, tag="eT_sb")
            nc.scalar.copy(out=vT_sb[:], in_=vT_ps[:])
            nc.scalar.copy(out=eT_sb[:], in_=eT_ps[:])

            d_ps = psD.tile([P, U * 16], f32, tag="d_ps")
            for j in range(U):
                b = j * 16
                nc.tensor.matmul(
                    d_ps[:, b : b + 8], vT_sb[:, j * P : (j + 1) * P], w_ab[:]
                )
                nc.tensor.matmul(
                    d_ps[:, b + 8 : b + 16], eT_sb[:, j * P : (j + 1) * P], w_ab[:]
                )
            d_sb = dsb.tile([P, U * 16], f32, tag="d_sb")
            nc.vector.tensor_copy(out=d_sb[:], in_=d_ps[:])

            o_ps = psO.tile([P, 2 * U * P], f32, tag="o_ps")
            for j in range(U):
                t1 = tmix.tile([P, 2, D], f32, tag="t1")
                t2 = tmix.tile([P, 2, D], f32, tag="t2")
                t3 = tmix.tile([P, 2, D], f32, tag="t3")
                t4 = tmix.tile([P, 2, D], f32, tag="t4")
                for u in range(2):
                    cv = j * 16 + u * 4
                    ce = cv + 8
                    nc.vector.tensor_scalar_mul(
                        t1[:, u], v_sb[:, j, u], d_sb[:, ce + 0 : ce + 1]
                    )
                    nc.vector.tensor_scalar_mul(
                        t2[:, u], e_sb[:, j, u], d_sb[:, cv + 2 : cv + 3]
                    )
                    nc.vector.tensor_scalar_mul(
                        t3[:, u], v_sb[:, j, u], d_sb[:, ce + 1 : ce + 2]
                    )
                    nc.vector.tensor_scalar_mul(
                        t4[:, u], e_sb[:, j, u], d_sb[:, cv + 3 : cv + 4]
                    )
                vsl = slice(j * P, (j + 1) * P)
                esl = slice(U * P + j * P, U * P + (j + 1) * P)
                nc.tensor.matmul(o_ps[:, vsl], ident[:], t1[:], start=True, stop=False)
                nc.tensor.matmul(o_ps[:, vsl], ident[:], t2[:], start=False, stop=False)
                nc.tensor.matmul(o_ps[:, esl], ident[:], t3[:], start=True, stop=False)
                nc.tensor.matmul(o_ps[:, esl], ident[:], t4[:], start=False, stop=False)
                nc.tensor.matmul(
                    o_ps[:, vsl], rowsel[:], biasrow_v[:], start=False, stop=True
                )
                nc.tensor.matmul(
                    o_ps[:, esl], rowsel[:], biasrow_e[:], start=False, stop=True
                )

            vo_sb = osb.tile([P, U, 2, D], f32, tag="vo_sb")
            eo_sb = osb.tile([P, U, 2, D], f32, tag="eo_sb")
            nc.scalar.copy(out=vo_sb[:], in_=o_ps[:, 0 : U * P])
            nc.scalar.copy(out=eo_sb[:], in_=o_ps[:, U * P : 2 * U * P])
            nc.scalar.dma_start(out=vout_r[g], in_=vo_sb[:])
            nc.scalar.dma_start(out=eout_r[g], in_=eo_sb[:])

    if split_waits:
        _split_sync_waits(nc)
    return nc


def _get_bass(rows: int, with_bias: bool):
    key = (rows, with_bias)
    if key not in _BUILD_CACHE:
        if with_bias:
            _BUILD_CACHE[key] = _build_bass_v1(rows, True)
        else:
            _BUILD_CACHE[key] = _build_v2(rows)
    return _BUILD_CACHE[key]


def kernel(v, e, w_vv, w_ev, w_ve, w_ee, bias_v, bias_e):
    global last_exec_time_ns, last_results
    from concourse.bass_utils import run_bass_kernel_spmd

    v = np.ascontiguousarray(np.asarray(v, dtype=np.float32))
    e = np.ascontiguousarray(np.asarray(e, dtype=np.float32))
    rows = v.shape[0]
    assert rows % N_CORES == 0
    shard = rows // N_CORES

    with_bias = bool(np.any(np.asarray(bias_v)) or np.any(np.asarray(bias_e)))
    nc = _get_bass(shard, with_bias)

    if with_bias:
        consts = {
            "w4": np.stack(
                [
                    np.asarray(w_vv, np.float32).reshape(D),
                    np.asarray(w_ve, np.float32).reshape(D),
                    np.asarray(w_ev, np.float32).reshape(D),
                    np.asarray(w_ee, np.float32).reshape(D),
                ]
            ),
            "bias2": np.stack(
                [
                    np.asarray(bias_v, np.float32).reshape(D),
                    np.asarray(bias_e, np.float32).reshape(D),
                ]
            ),
        }
    else:
        consts = {
            "w4t": np.stack(
                [
                    np.asarray(w_ev, np.float32).reshape(D),
                    np.asarray(w_vv, np.float32).reshape(D),
                    np.asarray(w_ee, np.float32).reshape(D),
                    np.asarray(w_ve, np.float32).reshape(D),
                ],
                axis=1,
            )
        }
    in_maps = []
    for i in range(N_CORES):
        m = dict(consts)
        m["v"] = v[i * shard : (i + 1) * shard]
        m["e"] = e[i * shard : (i + 1) * shard]
        in_maps.append(m)

    trace = os.environ.get("BASS_KERNEL_TRACE", "0") == "1"
    try:
        res = run_bass_kernel_spmd(
            nc, in_maps, core_ids=list(range(N_CORES)), trace=trace
        )
    except ModuleNotFoundError:
        res = run_bass_kernel_spmd(
            nc, in_maps, core_ids=list(range(N_CORES)), trace=False
        )
    last_exec_time_ns = res.exec_time_ns
    last_results = res

    v_out = np.concatenate([res.results[i]["v_out"] for i in range(N_CORES)], axis=0)
    e_out = np.concatenate([res.results[i]["e_out"] for i in range(N_CORES)], axis=0)
    return (v_out, e_out)
